# revision 1
# baseline (speedup 1.0000x reference)
"""Causal multi-head self-attention with RoPE on 8 Trainium2 NeuronCores.

Sharding: grid of 4 batches x 2 head-groups (8 heads each). Core c handles
batch c//2, heads (c%2)*8..(c%2)*8+8. Each core computes its partial output
projection (S, D); the host sums the two head-group partials per batch.

Weights are passed pre-transposed; wq/wk additionally have a per-head
even/odd de-interleave column permutation so RoPE on device is expressible
with contiguous 32-wide blocks. RoPE cos/sin are gathered host-side from
token_positions. All matmul operands are float32r (full-rate PE, ~1.5e-4
rel err). Softmax skips max-subtraction (logits are O(1) by construction);
denominators come free from an appended ones-column in V.
"""
import math
import numpy as np

import concourse.bass as bass
import concourse.mybir as mybir
import concourse.tile as tile
from concourse import bacc
from concourse.bass_utils import run_bass_kernel_spmd
from concourse.masks import make_identity

F32 = mybir.dt.float32
F32R = mybir.dt.float32r
BF16 = mybir.dt.bfloat16
EXP = mybir.ActivationFunctionType.Exp

D_MODEL = 1024
NUM_HEADS = 16
HEAD_DIM = 64
THETA = 10000.0
MAX_SEQ_LEN = 2048
BATCH = 4
S = 2048
H_LOC = 8           # heads per core
NI = 8              # contraction chunks of 128 over D_MODEL
ST = 16             # s-tiles of 128
QC = 4              # q-chunks of 512
N_CORES = 8

_PROGRAM_CACHE = {}


def _build_program(dup=1, phases=('a', 'att', 'proj')):
    nc = bacc.Bacc("TRN2", target_bir_lowering=False, debug=False,
                   num_devices=N_CORES)

    xT = nc.dram_tensor("xT", [D_MODEL, S], F32R, kind="ExternalInput")
    wqk = nc.dram_tensor("wqk", [D_MODEL, 1024], F32R, kind="ExternalInput")
    wv = nc.dram_tensor("wv", [D_MODEL, 512], F32R, kind="ExternalInput")
    woT = nc.dram_tensor("woT", [512, D_MODEL], F32R, kind="ExternalInput")
    cosD = nc.dram_tensor("cosD", [S, 64], F32, kind="ExternalInput")
    sinS = nc.dram_tensor("sinS", [S, 64], F32, kind="ExternalInput")
    out = nc.dram_tensor("out", [S, D_MODEL], F32, kind="ExternalOutput")

    with tile.TileContext(nc) as tc:
        with (
            tc.tile_pool(name="const", bufs=1) as cp,
            tc.tile_pool(name="stream", bufs=1) as sp,
            tc.tile_pool(name="psum", bufs=1, space="PSUM") as pp,
        ):
            # ---- resident constants -------------------------------------
            wqk_t = [cp.tile([128, 1024], F32R, tag="wqk", bufs=NI, name=f"wqk{i}")
                     for i in range(NI)]
            wv_t = [cp.tile([128, 512], F32R, tag="wv", bufs=NI, name=f"wv{i}")
                    for i in range(NI)]
            woT_t = [cp.tile([128, 1024], F32R, tag="woT", bufs=4, name=f"woT{i}")
                     for i in range(4)]
            for ic in range(NI):
                nc.sync.dma_start(wqk_t[ic][:], wqk[128 * ic:128 * (ic + 1), :])
                nc.sync.dma_start(wv_t[ic][:], wv[128 * ic:128 * (ic + 1), :])
            for p in range(4):
                nc.sync.dma_start(woT_t[p][:], woT[128 * p:128 * (p + 1), :])

            ident = cp.tile([128, 128], F32, tag="ident")
            make_identity(nc, ident[:])
            tri01 = cp.tile([128, 128], F32, tag="tri01")
            nc.gpsimd.memset(tri01[:], 1.0)
            nc.gpsimd.affine_select(      # keep q >= t, zero q < t
                out=tri01[:], in_=tri01[:], compare_op=mybir.AluOpType.is_ge,
                fill=0.0, base=0, pattern=[[1, 128]], channel_multiplier=-1)
            ones_f = cp.tile([128, 1], F32, tag="ones_f")
            nc.gpsimd.memset(ones_f[:], 1.0)
            ones_b = cp.tile([128, 1], BF16, tag="ones_b")
            nc.vector.tensor_copy(ones_b[:], ones_f[:])
            tri01b = cp.tile([128, 128], BF16, tag="tri01b")
            nc.vector.tensor_copy(tri01b[:], tri01[:])

            # kT / V' are resident for the whole kernel
            kT = [cp.tile([128, S], F32R, tag="kT", bufs=4, name=f"kT{i}")
                  for i in range(4)]
            Vp = [cp.tile([128, H_LOC * 65], BF16, tag="Vp", bufs=ST, name=f"Vp{i}")
                  for i in range(ST)]

            def stage_a(st, qT_cur):
                """projections + rope + transposes for s-tile st (128 rows)."""
                stL = st % 4
                xtb3 = sp.tile([128, NI, 128], F32R, tag="xtb", bufs=2, name=f"xtb{st}")
                nc.sync.dma_start(
                    xtb3[:],
                    xT[:].rearrange("(i p) s -> p i s", p=128)[:, :, 128 * st:128 * (st + 1)])
                xtb = [xtb3[:, ic, :] for ic in range(NI)]
                cos_t = sp.tile([128, 64], F32, tag="cos", bufs=2, name=f"cos{st}")
                sin_t = sp.tile([128, 64], F32, tag="sin", bufs=2, name=f"sin{st}")
                nc.sync.dma_start(cos_t[:], cosD[128 * st:128 * (st + 1), :])
                nc.sync.dma_start(sin_t[:], sinS[128 * st:128 * (st + 1), :])

                pq = pp.tile([128, 512], F32, tag="mm512", bufs=6, name=f"pq{st}")
                pk = pp.tile([128, 512], F32, tag="mm512", bufs=6, name=f"pk{st}")
                pv = pp.tile([128, 512], F32, tag="mm512", bufs=6, name=f"pv{st}")
                for ic in range(NI):
                    st_fl = dict(start=(ic == 0), stop=(ic == NI - 1))
                    nc.tensor.matmul(pq[:], xtb[ic], wqk_t[ic][:, 0:512], **st_fl)
                    nc.tensor.matmul(pk[:], xtb[ic], wqk_t[ic][:, 512:1024], **st_fl)
                    nc.tensor.matmul(pv[:], xtb[ic], wv_t[ic][:], **st_fl)

                # rope on q and k: out = q*cosD + swap(q)*sinS
                cos_b = (cos_t[:].rearrange("p (b i) -> p b i", b=2)
                         .unsqueeze(1).broadcast_to([128, 8, 2, 32]))
                sin_b = (sin_t[:].rearrange("p (b i) -> p b i", b=2)
                         .unsqueeze(1).broadcast_to([128, 8, 2, 32]))
                roped = []
                for psrc in (pq, pk):
                    p4 = psrc[:].rearrange("p (h b i) -> p h b i", h=8, b=2)
                    t1 = sp.tile([128, 512], F32, tag="ropet1", bufs=2, name=f"t1_{st}_{id(psrc)}")
                    nc.vector.tensor_mul(
                        t1[:].rearrange("p (h b i) -> p h b i", h=8, b=2),
                        p4[:, :, ::-1, :], sin_b)
                    t2 = sp.tile([128, 512], F32, tag="ropet2", bufs=2, name=f"t2_{st}_{id(psrc)}")
                    nc.vector.tensor_mul(
                        t2[:].rearrange("p (h b i) -> p h b i", h=8, b=2),
                        p4, cos_b)
                    r = sp.tile([128, 512], F32R, tag="roped", bufs=4, name=f"rope_{st}_{id(psrc)}")
                    nc.vector.tensor_add(r[:], t1[:], t2[:])
                    roped.append(r)
                qr, kr = roped

                for p in range(4):
                    ptr = pp.tile([128, 128], F32, tag="tr", bufs=2, name=f"ptrq{st}_{p}")
                    nc.tensor.transpose(ptr[:], qr[:, 128 * p:128 * (p + 1)].bitcast(F32), ident[:])
                    nc.vector.tensor_copy(qT_cur[p][:, 128 * stL:128 * (stL + 1)], ptr[:])
                    ptr2 = pp.tile([128, 128], F32, tag="tr", bufs=2, name=f"ptrk{st}_{p}")
                    nc.tensor.transpose(ptr2[:], kr[:, 128 * p:128 * (p + 1)].bitcast(F32), ident[:])
                    nc.vector.tensor_copy(kT[p][:, 128 * st:128 * (st + 1)], ptr2[:])

                # V' tile: [128 t, 8 heads x (64 v + ones)]
                v5 = Vp[st][:].rearrange("p (h c) -> p h c", h=H_LOC)
                nc.vector.tensor_copy(
                    v5[:, :, 0:64],
                    pv[:].rearrange("p (h c) -> p h c", h=H_LOC))
                nc.vector.tensor_copy(
                    v5[:, :, 64:65],
                    ones_b[:].unsqueeze(1).broadcast_to([128, H_LOC, 1]))

            def attention_pair(h0, qc, qT_cur, OT_cur, interleave=True):
                """Two heads (h0, h0+1) interleaved so PE has independent work
                while ACT runs the other head's exp."""
                hs = (h0, h0 + 1)
                pair = h0 // 2
                kTp = kT[pair]
                qTh = {h: qT_cur[pair][(h % 2) * 64:(h % 2) * 64 + 64, :] for h in hs}
                pot = {h: pp.tile([128, 512], F32, tag="mm512", bufs=6,
                                  name=f"pot{h}_{qc}") for h in hs}
                order = ([(tc, h) for tc in range(4 * qc) for h in hs] if interleave
                         else [(tc, h) for h in hs for tc in range(4 * qc)])
                for tc, h in order:                            # full tiles
                    if True:
                        sc = pp.tile([128, 512], F32, tag="mm512", bufs=6,
                                     name=f"sc{h}_{qc}_{tc}")
                        nc.tensor.matmul(
                            sc[:], kTp[(h % 2) * 64:(h % 2) * 64 + 64,
                                       128 * tc:128 * (tc + 1)], qTh[h])
                        pb = sp.tile([128, 512], BF16, tag="pt", bufs=6,
                                     name=f"pb{h}_{qc}_{tc}")
                        nc.scalar.activation(pb[:], sc[:], EXP, scale=0.125)
                        nc.tensor.matmul(
                            pot[h][0:65, :], Vp[tc][:, 65 * h:65 * (h + 1)], pb[:],
                            start=(tc == 0), stop=False)
                order_d = ([(i, h) for i in range(4) for h in hs] if interleave
                           else [(i, h) for h in hs for i in range(4)])
                for i, h in order_d:                           # diagonal region
                    tc = 4 * qc + i
                    if True:
                        sc = pp.tile([128, 512], F32, tag="mm512", bufs=6,
                                     name=f"scd{h}_{qc}_{i}")
                        nc.tensor.matmul(
                            sc[:, 128 * i:512],
                            kTp[(h % 2) * 64:(h % 2) * 64 + 64,
                                128 * tc:128 * (tc + 1)],
                            qTh[h][:, 128 * i:512])
                        pb = sp.tile([128, 512], BF16, tag="pt", bufs=6,
                                     name=f"pbd{h}_{qc}_{i}")
                        nc.scalar.activation(pb[:, 128 * i:512], sc[:, 128 * i:512],
                                             EXP, scale=0.125)
                        nc.vector.tensor_mul(pb[:, 128 * i:128 * (i + 1)],
                                             pb[:, 128 * i:128 * (i + 1)], tri01b[:])
                        nc.tensor.matmul(
                            pot[h][0:65, 128 * i:512], Vp[tc][:, 65 * h:65 * (h + 1)],
                            pb[:, 128 * i:512],
                            start=(qc == 0 and i == 0), stop=(i == 3))
                for h in hs:   # normalize by the ones-column sums (row 64)
                    rr = sp.tile([1, 512], F32, tag="rr", bufs=2, name=f"rr{h}_{qc}")
                    nc.vector.reciprocal(rr[:], pot[h][64:65, :])
                    bc = sp.tile([64, 512], F32, tag="bc", bufs=2, name=f"bc{h}_{qc}")
                    nc.gpsimd.partition_broadcast(bc[:], rr[:])
                    nc.vector.tensor_mul(OT_cur[pair][(h % 2) * 64:(h % 2) * 64 + 64, :],
                                         pot[h][0:64, :], bc[:])

            def projection(qc, OT_cur):
                for stL in range(4):
                    st = 4 * qc + stL
                    for half in range(2):
                        po = pp.tile([128, 512], F32, tag="mm512", bufs=6, name=f"po{qc}_{stL}_{half}")
                        for p in range(4):
                            nc.tensor.matmul(
                                po[:], OT_cur[p][:, 128 * stL:128 * (stL + 1)],
                                woT_t[p][:, 512 * half:512 * (half + 1)],
                                start=(p == 0), stop=(p == 3))
                        osb = sp.tile([128, 512], F32, tag="osb", bufs=4,
                                      name=f"osb{qc}_{stL}_{half}")
                        nc.scalar.copy(osb[:], po[:])
                        nc.sync.dma_start(
                            out[128 * st:128 * (st + 1), 512 * half:512 * (half + 1)],
                            osb[:])

            for rep in range(dup):
                for qc in range(QC):
                    qT_cur = [sp.tile([128, 512], F32R, tag="qT", bufs=8,
                                      name=f"qT{rep}_{qc}_{i}") for i in range(4)]
                    OT_cur = [sp.tile([128, 512], F32R, tag="OT", bufs=4,
                                      name=f"OT{rep}_{qc}_{i}") for i in range(4)]
                    if 'a' in phases:
                        for stL in range(4):
                            stage_a(4 * qc + stL, qT_cur)
                    if 'att' in phases:
                        for hp in range(4):
                            attention_pair(2 * hp, qc, qT_cur, OT_cur, interleave=False)
                    if 'proj' in phases:
                        projection(qc, OT_cur)

    nc.compile()
    return nc


def _get_program(dup=1, phases=('a', 'att', 'proj')):
    key = (dup, phases)
    if key not in _PROGRAM_CACHE:
        _PROGRAM_CACHE[key] = _build_program(dup, phases)
    return _PROGRAM_CACHE[key]


def _host_inputs(x, token_positions, wq, wk, wv, wo):
    x = np.asarray(x, dtype=np.float32)
    pos = np.asarray(token_positions)
    wq = np.asarray(wq, dtype=np.float32)
    wk = np.asarray(wk, dtype=np.float32)
    wv = np.asarray(wv, dtype=np.float32)
    wo = np.asarray(wo, dtype=np.float32)

    perm64 = np.concatenate([np.arange(0, 64, 2), np.arange(1, 64, 2)])
    # row selection for q/k with per-head de-interleave
    rows_perm = (np.arange(NUM_HEADS)[:, None] * 64 + perm64[None, :]).reshape(-1)
    wq_p = wq[rows_perm]            # (1024, 1024) permuted out-dims
    wk_p = wk[rows_perm]

    inv_freq = THETA ** (-np.arange(0, HEAD_DIM, 2, dtype=np.float32) / HEAD_DIM)
    ang = pos.astype(np.float32)[:, :, None] * inv_freq[None, None, :]  # (B,S,32)
    cosP = np.cos(ang, dtype=np.float32)
    sinP = np.sin(ang, dtype=np.float32)
    cosD = np.concatenate([cosP, cosP], axis=2)                  # (B,S,64)
    sinS = np.concatenate([-sinP, sinP], axis=2)

    in_maps = []
    for c in range(N_CORES):
        b, hg = c // 2, c % 2
        hsel = slice(512 * hg, 512 * (hg + 1))
        in_maps.append({
            "xT": np.ascontiguousarray(x[b].T),
            "wqk": np.ascontiguousarray(
                np.concatenate([wq_p[hsel].T, wk_p[hsel].T], axis=1)),
            "wv": np.ascontiguousarray(wv[hsel].T),
            "woT": np.ascontiguousarray(wo[:, hsel].T),
            "cosD": np.ascontiguousarray(cosD[b]),
            "sinS": np.ascontiguousarray(sinS[b]),
        })
    return in_maps


def kernel(x, token_positions, wq, wk, wv, wo, _trace=False):
    nc = _get_program()
    in_maps = _host_inputs(x, token_positions, wq, wk, wv, wo)
    res = run_bass_kernel_spmd(nc, in_maps, core_ids=list(range(N_CORES)),
                               trace=_trace)
    parts = [r["out"] for r in res.results]
    out = np.stack([parts[2 * b] + parts[2 * b + 1] for b in range(BATCH)])
    kernel._last_result = res
    return out.astype(np.float32)

